# revision 55
# baseline (speedup 1.0000x reference)
"""Binarized ResNet BasicBlock (conv1 3x3/s2 + BN + sign, conv2 3x3 + BN,
1x1/s2 shortcut conv + BN, add, sign) as a Bass/Tile kernel on 8 TRN2 cores.

Strategy (470us baseline -> ~341us, rel err ~6e-3 vs the 2e-2 gate):
- The shortcut runs as ONE fp32r pass on raw fp32 x (measured: ~13
  effective mantissa bits at full 1-cycle/row rate for moving dim >=
  256). Its error feeds the final sign directly with no conv2 fan-out,
  costing only ~1e-5 of sign flips (129, deterministic); halves the
  shortcut matmuls. conv1 cannot use fp32r: it needs ~20 bits (see
  below) and the fp32r rounding is hw-internal, so no host-side
  residual pass can correct it.
- Data-parallel over batch: 16 images per core, weights/BN params replicated.
- Binarized weights are exactly +-1 in fp8. x is split HOST-SIDE into 2
  fp16 terms (hi = fp16(x), lo = fp16(x - hi)) whose products with +-1
  fp8 weights are exact (PE handles fp16 subnormals exactly; mixed fp8
  lhsT x fp16 rhs runs at full rate), so conv1 and the shortcut
  accumulate x to ~2^-22 relative in fp32 PSUM. ~20 bits are required:
  an a1 sign-flip fans out through conv2's 4608-wide receptive fields,
  amplifying the final flip rate ~100x.
- conv2's inputs are +-1: both operands are fp8e4 and the matmuls run in
  DoubleRow perf mode (2 fp8 weights per PE cell, 256-deep contraction
  per pass) — exact integer arithmetic measured at the fp8 roofline
  (~173ns per 392-output-column matmul = 157 TF/s). a1 is stored as
  [128, ksub=2, ...] pair tiles so each DoubleRow matmul consumes two
  128-channel blocks at once; w2 is laid out [128, tap*2+pair, ksub,
  cout]; the per-image stride is 224B to keep the ksub step 16B-aligned.
- sign(clip(bn(z))) == sign(bn(z)): fused into one Sign activation with
  per-channel scale/bias APs.
- Consecutive matmuls never reuse a stationary weight tile (same-weight
  back-to-back serializes LDWEIGHTS; rotating weights pipelines it away).
- Conv taps land at tap-dependent PSUM offsets (per-element has_written
  gives overwrite-then-accumulate): x is packed as stride-2 parity planes
  so conv1 taps read contiguous (kw=1) or column-trimmed strided (kw=0,2,
  dropping the zero-pad / unused columns) spans; conv2 streams row-trimmed
  unpadded a1 images (kh=0/2 taps drop the row that would land in the
  garbage border) into a 16x16 PSUM window. The remaining column-trim
  would need a true 5D moving AP, which the backend rejects.
- PSUM tiles are always allocated at full group size: a tile smaller than
  the 2KB bank would share a bank with a neighbor, and a start=True
  matmul clears the whole bank.
- Startup: DMA descriptor issues serialize at ~650ns on the Sync engine,
  so loads are few and large, emitted in first-use order (x-hi, the
  ci-major first half of w1, ...). Single-image warm-up/tail groups were
  tried and measured slower (LDW-bound small matmuls cost more than the
  pacing they buy); the schedule is uniform 2-image groups.
"""

import numpy as np
import ml_dtypes
from contextlib import ExitStack

import concourse.tile as tile
from concourse import mybir, bacc
from concourse.bass_utils import run_bass_kernel_spmd

bf16 = ml_dtypes.bfloat16
f8e4 = ml_dtypes.float8_e4m3
F32 = mybir.dt.float32
F32R = mybir.dt.float32r
BF = mybir.dt.bfloat16
F16 = mybir.dt.float16
F8 = mybir.dt.float8e4
SIGN = mybir.ActivationFunctionType.Sign
IDENT = mybir.ActivationFunctionType.Identity
DR = mybir.MatmulPerfMode.DoubleRow

N_CORES = 8
B, CIN, COUT, H = 128, 256, 512, 28
OH = 14                      # output spatial
BPC = B // N_CORES           # images per core
G = 2                        # images per matmul group
NG = BPC // G                # groups per core
NPG = G * OH * OH            # 392 valid pixels per group
NCT = COUT // 128            # cout tiles (4)
NCI1 = CIN // 128            # cin tiles for conv1/shortcut (2)
NCI2 = COUT // 128           # cin tiles for conv2 (4)
NSPL = 2                     # split terms for x (fp16 hi/lo)
ISTR = 224                   # a1 per-image stride (fp8 bytes, 16-aligned)
EPS = np.float32(1e-5)

# parity-plane packing of the 29x29 zero-padded input (pad at index 0):
# plane (ph, pw) = xpad[2i+ph, 2j+pw]; heights/widths 15 or 14.
PL_W = {0: 15, 1: 14}  # plane widths by w-parity (heights analogous)
PL_OFF = {(0, 0): 0, (0, 1): 225, (1, 0): 435, (1, 1): 645}

_prog_cache = {}


def _build_program():
    nc = bacc.Bacc("TRN2", debug=False)

    # host-pre-split x: [128, 2 (hi/lo), BPC, 841] fp16 parity planes
    xs = [nc.dram_tensor(f"xs{ci}", [128, NSPL, BPC, 841], F16,
                         kind="ExternalInput").ap() for ci in range(NCI1)]
    # +-1 weights are exact in fp8; lhsT fp8 with fp16 rhs halves the
    # startup-critical weight DMA
    w1 = nc.dram_tensor("w1t", [128, 9 * NCI1, COUT], F8, kind="ExternalInput").ap()
    w2 = nc.dram_tensor("w2t", [128, 18, 2, COUT], F8, kind="ExternalInput").ap()
    wsc = nc.dram_tensor("wsct", [128, NCI1, COUT], F8, kind="ExternalInput").ap()
    # single-pass fp32r shortcut for 2-image groups: ~13 effective mantissa
    # bits (measured) feed the final sign directly (no conv2 flip
    # amplification), trading ~1e-5 of sign flips for half the sc matmuls
    wsr = nc.dram_tensor("wsrt", [128, NCI1, COUT], F32R, kind="ExternalInput").ap()
    xq = nc.dram_tensor("xqt", [128, NCI1, BPC, 196], F32R,
                        kind="ExternalInput").ap()
    bnc = nc.dram_tensor("bnc", [128, 5, NCT], F32, kind="ExternalInput").ap()
    y = nc.dram_tensor("y", [128, NCT, BPC, OH * OH], BF,
                       kind="ExternalOutput").ap()

    with tile.TileContext(nc) as tc, ExitStack() as ctx:
        consts = ctx.enter_context(tc.tile_pool(name="consts", bufs=1))
        spl = ctx.enter_context(tc.tile_pool(name="spl", bufs=6))
        xqp = ctx.enter_context(tc.tile_pool(name="xqp", bufs=4))
        a1p = ctx.enter_context(tc.tile_pool(name="a1p", bufs=6))
        yp = ctx.enter_context(tc.tile_pool(name="yp", bufs=3))
        up = ctx.enter_context(tc.tile_pool(name="up", bufs=6))
        pA = ctx.enter_context(tc.tile_pool(name="pA", bufs=4, space="PSUM"))
        pS = ctx.enter_context(tc.tile_pool(name="pS", bufs=4, space="PSUM"))

        # w1/wsc/bnc load first (needed by group 0); w2's DMA is emitted after
        # group 0's x load so it doesn't block startup.
        w1_sb = consts.tile([128, 9 * NCI1, COUT], F8)
        wsc_sb = consts.tile([128, NCI1, COUT], F8)
        wsr_sb = consts.tile([128, NCI1, COUT], F32R)
        bnc_sb = consts.tile([128, 5, NCT], F32)
        w2_sb = consts.tile([128, 18, 2, COUT], F8)

        def bn_ap(i, c):
            return bnc_sb[:, i, c:c + 1]

        # uniform 2-image groups: single-image warm-up/tail groups were
        # measured to cost more in LDW-bound small matmuls (~1.2us/image)
        # than they saved in DMA pacing or tail latency
        sched = [(b0, G) for b0 in range(0, BPC, G)]
        for gi, (b0, gs) in enumerate(sched):
            bsl = slice(b0, b0 + gs)
            npg = gs * OH * OH
            # ---- load pre-split hi/lo fp16 parity planes ----
            parts = []  # parts[ci] = tile [128, 2, G, 841]
            for ci in range(NCI1):
                xt = spl.tile([128, NSPL, G, 841], F16, tag="spl",
                              name=f"x_{gi}_{ci}")
                if gi == 0:
                    # hi before w1 before lo, in first-use order
                    nc.sync.dma_start(xt[:, 0, 0:gs], xs[ci][:, 0, bsl])
                    if ci == 0:
                        # ci-major block layout: the ci=0 half is all the
                        # first 9 matmuls need, so split the load in two
                        nc.sync.dma_start(w1_sb[:, 0:9], w1[:, 0:9])
                        nc.sync.dma_start(w1_sb[:, 9:18], w1[:, 9:18])
                    nc.sync.dma_start(xt[:, 1, 0:gs], xs[ci][:, 1, bsl])
                    if ci == NCI1 - 1:
                        nc.sync.dma_start(wsc_sb[:], wsc[:])
                        nc.sync.dma_start(bnc_sb[:], bnc[:])
                else:
                    nc.sync.dma_start(xt[:, :, 0:gs], xs[ci][:, :, bsl])
                parts.append(xt)
            if gi == 0:
                # must be emitted before group 0's conv2 matmuls (the dep
                # tracker only orders reads after earlier-emitted writes)
                nc.sync.dma_start(w2_sb[:], w2[:])
                nc.sync.dma_start(wsr_sb[:], wsr[:])
            xq_t = None
            if gs == G:
                # raw fp32 plane-11 for the single-pass fp32r shortcut
                xq_t = xqp.tile([128, NCI1, G, 196], F32R, tag="xq",
                                name=f"xq_{gi}")
                nc.sync.dma_start(xq_t[:, :, 0:gs], xq[:, :, bsl])

            # ---- conv1 + interleaved shortcut matmuls ----
            # conv1 psum [128, gs, 14, 16]; valid cols 1..14
            p1, psc = [], []
            for c in range(NCT):
                # always full-G size: a psum tile smaller than the 2KB bank
                # would share a bank with its neighbor, and a start=True
                # matmul clears the whole bank
                pt = pA.tile([128, G, 14, 16], F32, tag="pA", name=f"p1_{gi}_{c}")
                pt = pt[:, 0:gs]
                idx, last = 0, NCI1 * 9 * NSPL - 1
                for s in range(NSPL):
                    for ci in range(NCI1):
                        for t in range(9):
                            kh, kw = divmod(t, 3)
                            ph, pw = kh & 1, kw & 1
                            w_pl = PL_W[pw]
                            dh = 1 if kh == 2 else 0
                            # kh==0 taps read plane row 0 = the zero pad row;
                            # skip it (contributes exact zeros) -> 13 rows
                            r0 = 1 if kh == 0 else 0
                            nrows = 14 - r0
                            off = PL_OFF[(ph, pw)] + (dh + r0) * w_pl
                            w_ap = w1_sb[:, ci * 9 + t, c * 128:(c + 1) * 128]
                            if kw == 1:
                                # pw=1 plane: all 14 cols are real data
                                rhs = parts[ci][:, s, 0:gs,
                                                off:off + nrows * w_pl]
                                out = pt[:, :, r0:14, 1:15]
                            else:
                                # pw=0 planes: col 0 is the zero-pad col and
                                # (for kw=0) col 14 is unused; stream a
                                # column-trimmed strided window instead
                                cs, w, c0 = (1, 13, 2) if kw == 0 else (1, 14, 1)
                                rhs = parts[ci][:, s, 0:gs,
                                                off:off + nrows * w_pl].rearrange(
                                    "p g (r w) -> p g r w", r=nrows, w=w_pl)[
                                    :, :, :, cs:cs + w]
                                out = pt[:, :, r0:14, c0:c0 + w]
                            nc.tensor.matmul(out, w_ap, rhs,
                                             start=(idx == 0), stop=(idx == last))
                            idx += 1
                p1.append(pt)
                # shortcut for this cout tile: odd/odd parity plane
                st = pS.tile([128, NPG], F32, tag="pS", name=f"psc_{gi}_{c}")
                st = st[:, 0:npg]
                if xq_t is not None:
                    # single fp32r pass (full-rate needs moving dim >= 256,
                    # so only for 2-image groups)
                    for ci in range(NCI1):
                        w_ap = wsr_sb[:, ci, c * 128:(c + 1) * 128]
                        nc.tensor.matmul(st[:, 0:npg], w_ap,
                                         xq_t[:, ci, 0:gs],
                                         start=(ci == 0),
                                         stop=(ci == NCI1 - 1))
                else:
                    idx, last = 0, NCI1 * NSPL - 1
                    for s in range(NSPL):
                        for ci in range(NCI1):
                            w_ap = wsc_sb[:, ci, c * 128:(c + 1) * 128]
                            rhs = parts[ci][:, s, 0:gs, 645:841]
                            nc.tensor.matmul(st[:, 0:npg], w_ap, rhs,
                                             start=(idx == 0),
                                             stop=(idx == last))
                            idx += 1
                psc.append(st)

            # ---- a1 = sign(bn1(conv1)), fp8 +-1, [128, ksub, G, 224] ----
            # pair tile pr holds conv1 cout tiles {2pr, 2pr+1} as the two
            # DoubleRow k-subtiles; per-image stride 224 keeps the ksub
            # step 16B-aligned (bytes 196:224 per image are garbage).
            a1q = []
            for pr in range(2):
                a1q.append(a1p.tile([128, 2, G, ISTR], F8, tag="a1",
                                    name=f"a1_{gi}_{pr}"))
            for c in range(NCT):
                dst = a1q[c // 2][:, c % 2, 0:gs, 0:196].rearrange(
                    "p g (h w) -> p g h w", h=OH, w=OH)
                nc.scalar.activation(dst, p1[c][:, :, :, 1:15],
                                     SIGN, bias=bn_ap(1, c), scale=bn_ap(0, c))

            # ---- conv2: fp8 DoubleRow, whole-a1 streams into shifted
            # 16x16 psum windows; valid [1:15, 1:15] ----
            p2 = []
            for c in range(NCT):
                pt = pA.tile([128, G, 16, 16], F32, tag="pA", name=f"p2_{gi}_{c}")
                pt = pt[:, 0:gs]
                idx, last = 0, 17
                for pr in range(2):
                    for t in range(9):
                        kh, kw = divmod(t, 3)
                        w_ap = w2_sb[:, t * 2 + pr, :, c * 128:(c + 1) * 128]
                        # trim each tap's stream to the a1 ROWS whose
                        # output lands in the valid [1:15] psum rows
                        # (kh=0/2 taps otherwise waste a row per image on
                        # garbage-border writes); whole rows keep the rhs
                        # span contiguous, which the backend requires
                        r0, nr = {0: (0, 13), 1: (0, 14), 2: (1, 13)}[kh]
                        rhs = a1q[pr][:, :, 0:gs,
                                      r0 * OH:(r0 + nr) * OH].rearrange(
                            "p k g (h w) -> p k g h w", h=nr, w=OH)
                        out = pt[:, :, 2 - kh + r0:2 - kh + r0 + nr,
                                 2 - kw:16 - kw]
                        nc.tensor.matmul(out, w_ap, rhs, perf_mode=DR,
                                         start=(idx == 0), stop=(idx == last))
                        idx += 1
                p2.append(pt)

            # ---- y = sign(scale2*p2 + shift2 + scalesc*psc + shiftsc) ----
            yt = yp.tile([128, NCT, NPG], BF, tag="y", name=f"y_{gi}")
            for c in range(NCT):
                # wt = scale2*p2 + (shift2+shiftsc): ACT handles the 4D
                # strided psum window; stt only takes 2D/3D operands.
                wt = up.tile([128, NPG], F32, tag="u", name=f"u_{gi}_{c}")
                nc.scalar.activation(
                    wt[:, 0:npg].rearrange("p (b h w) -> p b h w",
                                           b=gs, h=OH, w=OH),
                    p2[c][:, :, 1:15, 1:15], IDENT,
                    bias=bn_ap(3, c), scale=bn_ap(2, c))
                vt = up.tile([128, NPG], F32, tag="v", bufs=3, name=f"v_{gi}_{c}")
                nc.vector.scalar_tensor_tensor(
                    vt[:, 0:npg], psc[c][:, 0:npg], bn_ap(4, c), wt[:, 0:npg],
                    op0=mybir.AluOpType.mult, op1=mybir.AluOpType.add)
                nc.scalar.activation(yt[:, c, 0:npg], vt[:, 0:npg], SIGN)
                # per-couttile DMA so the tail drains as soon as each
                # couttile's sign completes
                nc.sync.dma_start(
                    y[:, c, bsl].rearrange("p b x -> p (b x)"),
                    yt[:, c, 0:npg])

    nc.compile()
    return nc


def _prep_consts(w1, w2, wsc, g1, b1, m1, v1, g2, b2, m2, v2, gsc, bsc, msc, vsc):
    def sgn_w(w, dt):
        return np.where(w >= 0, np.float32(1.0), np.float32(-1.0)).astype(dt)

    # conv1 lhsT layout: [cin_part(128), ci*9+tap, cout] (ci-major so the
    # first-used half of the weights is one contiguous DMA)
    w1s = sgn_w(w1, f8e4)  # [COUT, CIN, 3, 3]
    a1w = np.empty((128, 9 * NCI1, COUT), f8e4)
    for t in range(9):
        kh, kw = divmod(t, 3)
        for ci in range(NCI1):
            a1w[:, ci * 9 + t, :] = w1s[:, ci * 128:(ci + 1) * 128, kh, kw].T
    # conv2 DoubleRow lhsT layout: [cin_part(128), tap*2+pair, ksub, cout]
    # where pair pr covers cin channels 256pr..256pr+255, ksub within pair.
    w2s = sgn_w(w2, f8e4)
    a2w = np.empty((128, 18, 2, COUT), f8e4)
    for t in range(9):
        kh, kw = divmod(t, 3)
        for pr in range(2):
            for ks in range(2):
                ci = pr * 2 + ks
                a2w[:, t * 2 + pr, ks, :] = \
                    w2s[:, ci * 128:(ci + 1) * 128, kh, kw].T
    wscs = sgn_w(wsc, f8e4)
    asw = np.empty((128, NCI1, COUT), f8e4)
    asr = np.empty((128, NCI1, COUT), np.float32)
    for ci in range(NCI1):
        asw[:, ci, :] = wscs[:, ci * 128:(ci + 1) * 128, 0, 0].T
        asr[:, ci, :] = asw[:, ci, :].astype(np.float32)

    def bn_affine(g, b, m, v):
        scale = (g / np.sqrt(v + EPS)).astype(np.float32)
        shift = (b - m * g / np.sqrt(v + EPS)).astype(np.float32)
        return scale, shift

    sc1, sh1 = bn_affine(g1, b1, m1, v1)
    sc2, sh2 = bn_affine(g2, b2, m2, v2)
    scs, shs = bn_affine(gsc, bsc, msc, vsc)
    bnc = np.empty((128, 5, NCT), np.float32)
    for c in range(NCT):
        cs = slice(c * 128, (c + 1) * 128)
        bnc[:, 0, c] = sc1[cs]
        bnc[:, 1, c] = sh1[cs]
        bnc[:, 2, c] = sc2[cs]
        bnc[:, 3, c] = (sh2 + shs)[cs]
        bnc[:, 4, c] = scs[cs]
    return a1w, a2w, asw, asr, bnc


def kernel(x, w1, g1, b1, m1, v1, w2, g2, b2, m2, v2, wsc, gsc, bsc, msc, vsc,
           _trace=False):
    x = np.ascontiguousarray(x, np.float32)
    a1w, a2w, asw, asr, bnc = _prep_consts(
        np.asarray(w1, np.float32), np.asarray(w2, np.float32),
        np.asarray(wsc, np.float32),
        *[np.asarray(t, np.float32) for t in (g1, b1, m1, v1)],
        *[np.asarray(t, np.float32) for t in (g2, b2, m2, v2)],
        *[np.asarray(t, np.float32) for t in (gsc, bsc, msc, vsc)])

    # padded, channel-major x repacked as concatenated stride-2 parity planes
    xpad = np.zeros((CIN, B, H + 1, H + 1), np.float32)
    xpad[:, :, 1:, 1:] = x.transpose(1, 0, 2, 3)
    xflat = np.concatenate(
        [xpad[:, :, ph::2, pw::2].reshape(CIN, B, -1)
         for ph in (0, 1) for pw in (0, 1)], axis=2)  # [CIN, B, 841]
    # exact host-side 2-term fp16 split
    xhi = xflat.astype(np.float16)
    xlo = (xflat - xhi.astype(np.float32)).astype(np.float16)
    xsp = np.stack([xhi, xlo], axis=1)  # [CIN, 2, B, 841]

    if "nc" not in _prog_cache:
        _prog_cache["nc"] = _build_program()
    nc = _prog_cache["nc"]

    xq = xflat[:, :, 645:841]  # raw fp32 plane-11 for the fp32r shortcut
    in_maps = []
    for k in range(N_CORES):
        m = {"w1t": a1w, "w2t": a2w, "wsct": asw, "wsrt": asr, "bnc": bnc}
        for ci in range(NCI1):
            m[f"xs{ci}"] = np.ascontiguousarray(
                xsp[ci * 128:(ci + 1) * 128, :, k * BPC:(k + 1) * BPC])
        m["xqt"] = np.ascontiguousarray(np.stack(
            [xq[ci * 128:(ci + 1) * 128, k * BPC:(k + 1) * BPC]
             for ci in range(NCI1)], axis=1))
        in_maps.append(m)

    res = run_bass_kernel_spmd(nc, in_maps, core_ids=list(range(N_CORES)),
                               trace=_trace)

    # y dram: [128, NCT, BPC, 196] per core -> [B, COUT, 14, 14]
    out = np.empty((B, COUT, OH, OH), np.float32)
    for k in range(N_CORES):
        yk = np.asarray(res.results[k]["y"], np.float32)  # [128, 4, 16, 196]
        out[k * BPC:(k + 1) * BPC] = (
            yk.transpose(2, 1, 0, 3).reshape(BPC, COUT, OH, OH))
    if _trace:
        kernel.last_results = res
    return out
